# revision 10
# baseline (speedup 1.0000x reference)
"""CenterLoss kernel for Trainium2 (8 NeuronCores, data-parallel over batch).

reference: mean(clip(distmat[i, labels[i]])) where
  distmat[i,c] = ||x_i||^2 + ||c_c||^2 - 2 x_i . c_c
i.e. the loss only needs dist_i = ||x_i - centers[labels[i]]||^2 — a gather +
elementwise + reduce; the full (N, C) matmul in the reference is dead work.

Per core (512 rows of the 4096-row batch), bf16 data path (inputs quantized
host-side; final rel err ~1e-5, far under the 2e-2 gate):
  - labels enter as [128, 4] int32; a single indirect-DMA gather (SWDGE) pulls
    all 512 center rows into a [128, 4*512] tile. One instruction: SWDGE
    desc-gen is 994ns fixed + 0.34ns/desc, so batching all 512 descriptors
    amortizes the fixed cost (4 chunks would pay it 4x).
  - x shard enters SBUF as [128, 4*512] in one HWDGE DMA on the scalar ring.
  - VectorE: diff = x - c (bf16, 2x DVE rate); ScalarE: Square activation with
    accum_out gives the per-row sum of squares in f32 directly.
  - [128, 4] per-row f32 distances DMA out; host concatenates 8 cores, applies
    the clip (a no-op for this data but kept for exactness) and the mean.
"""

import os

import numpy as np
import ml_dtypes

# clears a wedged NeuronCore from a previous crashed run at NRT init
os.environ.setdefault("NEURON_RT_RESET_CORES", "1")

N, D, C = 4096, 512, 10000
NCORES = 8
ROWS_PER_CORE = N // NCORES  # 512
P = 128
J = ROWS_PER_CORE // P  # 4 rows per partition

CLAMP = 1e-12

_cache = {}

# tuning knobs
GATHER_SPLIT = [2, 2]  # label columns per indirect-gather instruction
LABELS_ENGINE = "sync"  # queue for the labels DMA
X_ENGINE = "scalar"  # queue for the x DMA
X_CHUNKS = 1  # x DMAs
# per-column compute plan: engine pair (sub, square+accum) for col j.
#   sub: "v" = DVE tensor_tensor, "p" = GpSimd tensor_tensor
#   red: "v" = DVE scalar_tensor_tensor accum, "a" = ScalarE Square activation
# GpSimd compute measured 4x slower and contends with DVE for SBUF ports —
# keep subs on DVE; split reduces between ScalarE (early cols) and DVE.
COMPUTE_PLAN = ["va", "vv", "va", "vv"]


def _build_nc():
    import concourse.bass as bass
    import concourse.mybir as mybir
    from concourse import bacc
    from concourse.tile import TileContext

    nc = bacc.Bacc(
        "TRN2",
        target_bir_lowering=False,
        debug=False,
        num_devices=NCORES,
        # default 16KB ring stalls Q7: 512 gather descriptors x 64B = 32KB
        dynamic_dma_scratch_size=65536,
    )
    bf16 = mybir.dt.bfloat16
    x = nc.dram_tensor("x", [P, J * D], bf16, kind="ExternalInput")
    labels = nc.dram_tensor("labels", [P, J], mybir.dt.int32, kind="ExternalInput")
    centers = nc.dram_tensor("centers", [C, D], bf16, kind="ExternalInput")
    out = nc.dram_tensor("out", [P, J], mybir.dt.float32, kind="ExternalOutput")

    assert sum(GATHER_SPLIT) == J

    with TileContext(nc) as tc:
        with (
            tc.tile_pool(name="io", bufs=1) as io_pool,
            tc.tile_pool(name="work", bufs=J) as work,
        ):
            # labels first — the gather is gated on it
            lab_tile = io_pool.tile([P, J], mybir.dt.int32)
            getattr(nc, LABELS_ENGINE).dma_start(out=lab_tile[:], in_=labels[:])

            # gather instruction(s) right behind labels on the Pool queue
            g_tile = io_pool.tile([P, J * D], bf16)
            lo = 0
            for g in GATHER_SPLIT:
                hi = lo + g
                nc.gpsimd.indirect_dma_start(
                    out=g_tile[:, lo * D : hi * D],
                    out_offset=None,
                    in_=centers[:],
                    in_offset=bass.IndirectOffsetOnAxis(
                        ap=lab_tile[:, lo:hi], axis=0
                    ),
                )
                lo = hi

            # x on an HW ring, in parallel with labels
            x_tile = io_pool.tile([P, J * D], bf16)
            x_eng = getattr(nc, X_ENGINE)
            cw = J // X_CHUNKS * D
            for i in range(X_CHUNKS):
                x_eng.dma_start(
                    out=x_tile[:, i * cw : (i + 1) * cw],
                    in_=x[:, i * cw : (i + 1) * cw],
                )

            dists = io_pool.tile([P, J], mybir.dt.float32)

            for j, plan in enumerate(COMPUTE_PLAN):
                sub_eng = nc.gpsimd if plan[0] == "p" else nc.vector
                diff = work.tile([P, D], bf16, tag="d")
                sub_eng.tensor_tensor(
                    out=diff[:],
                    in0=x_tile[:, j * D : (j + 1) * D],
                    in1=g_tile[:, j * D : (j + 1) * D],
                    op=mybir.AluOpType.subtract,
                )
                sq = work.tile([P, D], bf16, tag="s")
                if plan[1] == "p":
                    raise ValueError("TensorScalarPtr unsupported on Pool")
                if plan[1] == "a":
                    nc.scalar.activation(
                        out=sq[:],
                        in_=diff[:],
                        func=mybir.ActivationFunctionType.Square,
                        accum_out=dists[:, j : j + 1],
                    )
                else:
                    nc.vector.scalar_tensor_tensor(
                        out=sq[:],
                        in0=diff[:],
                        scalar=0.0,
                        in1=diff[:],
                        op0=mybir.AluOpType.add,
                        op1=mybir.AluOpType.mult,
                        accum_out=dists[:, j : j + 1],
                    )

            nc.sync.dma_start(out=out[:], in_=dists[:])

    nc.compile()
    return nc


def _run(in_maps, trace=False):
    from concourse.bass_utils import run_bass_kernel_spmd

    if "nc" not in _cache:
        _cache["nc"] = _build_nc()
    return run_bass_kernel_spmd(
        _cache["nc"], in_maps, list(range(NCORES)), trace=trace
    )


def kernel(x, labels, centers, _trace=False):
    x = np.ascontiguousarray(np.asarray(x)).astype(ml_dtypes.bfloat16)
    labels = np.asarray(labels).astype(np.int32)
    centers = np.ascontiguousarray(np.asarray(centers)).astype(ml_dtypes.bfloat16)

    R = ROWS_PER_CORE
    in_maps = []
    for c in range(NCORES):
        lo = c * R
        hi = lo + R
        in_maps.append(
            {
                "x": x[lo:hi].reshape(P, J * D),
                "labels": np.ascontiguousarray(labels[lo:hi].reshape(P, J)),
                "centers": centers,
            }
        )

    res = _run(in_maps, trace=_trace)
    dists = np.concatenate(
        [res.results[c]["out"].reshape(R) for c in range(NCORES)]
    )
    loss = np.clip(dists, CLAMP, 1.0 / CLAMP).mean(dtype=np.float64)
    out = np.asarray(loss, dtype=np.float32)
    if _trace:
        return out, res
    return out


# revision 11
# speedup vs baseline: 1.0765x; 1.0765x over previous
"""CenterLoss kernel for Trainium2 (8 NeuronCores, data-parallel over batch).

reference: mean(clip(distmat[i, labels[i]])) where
  distmat[i,c] = ||x_i||^2 + ||c_c||^2 - 2 x_i . c_c
i.e. the loss only needs dist_i = ||x_i - centers[labels[i]]||^2 — a gather +
elementwise + reduce; the full (N, C) matmul in the reference is dead work.

Per core (512 rows of the 4096-row batch), bf16 data path (inputs quantized
host-side; final rel err ~1e-5, far under the 2e-2 gate):
  - labels enter as [128, 4] int32; a single indirect-DMA gather (SWDGE) pulls
    all 512 center rows into a [128, 4*512] tile. One instruction: SWDGE
    desc-gen is 994ns fixed + 0.34ns/desc, so batching all 512 descriptors
    amortizes the fixed cost (4 chunks would pay it 4x).
  - x shard enters SBUF as [128, 4*512] in one HWDGE DMA on the scalar ring.
  - VectorE: diff = x - c (bf16, 2x DVE rate); ScalarE: Square activation with
    accum_out gives the per-row sum of squares in f32 directly.
  - [128, 4] per-row f32 distances DMA out; host concatenates 8 cores, applies
    the clip (a no-op for this data but kept for exactness) and the mean.
"""

import os

import numpy as np
import ml_dtypes

# clears a wedged NeuronCore from a previous crashed run at NRT init
os.environ.setdefault("NEURON_RT_RESET_CORES", "1")

N, D, C = 4096, 512, 10000
NCORES = 8
ROWS_PER_CORE = N // NCORES  # 512
P = 128
J = ROWS_PER_CORE // P  # 4 rows per partition

CLAMP = 1e-12

_cache = {}

# tuning knobs
GATHER_SPLIT = [2, 2]  # label columns per indirect-gather instruction
# labels and x share the sync HWDGE ring, labels first: queue FIFO keeps the
# 512KB x transfer from delaying the tiny labels DMA that gates the gather.
LABELS_ENGINE = "sync"  # queue for the labels DMA
X_ENGINE = "sync"  # queue for the x DMA
X_CHUNKS = 1  # x DMAs
# per-column compute plan: engine pair (sub, square+accum) for col j.
#   sub: "v" = DVE tensor_tensor, "p" = GpSimd tensor_tensor
#   red: "v" = DVE scalar_tensor_tensor accum, "a" = ScalarE Square activation
# GpSimd compute measured 4x slower and contends with DVE for SBUF ports —
# keep subs on DVE; split reduces between ScalarE (early cols) and DVE.
COMPUTE_PLAN = ["va", "vv", "va", "vv"]


def _build_nc():
    import concourse.bass as bass
    import concourse.mybir as mybir
    from concourse import bacc
    from concourse.tile import TileContext

    nc = bacc.Bacc(
        "TRN2",
        target_bir_lowering=False,
        debug=False,
        num_devices=NCORES,
        # default 16KB ring stalls Q7: 512 gather descriptors x 64B = 32KB
        dynamic_dma_scratch_size=65536,
    )
    bf16 = mybir.dt.bfloat16
    x = nc.dram_tensor("x", [P, J * D], bf16, kind="ExternalInput")
    labels = nc.dram_tensor("labels", [P, J], mybir.dt.int32, kind="ExternalInput")
    centers = nc.dram_tensor("centers", [C, D], bf16, kind="ExternalInput")
    out = nc.dram_tensor("out", [P, J], mybir.dt.float32, kind="ExternalOutput")

    assert sum(GATHER_SPLIT) == J

    with TileContext(nc) as tc:
        with (
            tc.tile_pool(name="io", bufs=1) as io_pool,
            tc.tile_pool(name="work", bufs=J) as work,
        ):
            # labels first — the gather is gated on it
            lab_tile = io_pool.tile([P, J], mybir.dt.int32)
            getattr(nc, LABELS_ENGINE).dma_start(out=lab_tile[:], in_=labels[:])

            # gather instruction(s) right behind labels on the Pool queue
            g_tile = io_pool.tile([P, J * D], bf16)
            lo = 0
            for g in GATHER_SPLIT:
                hi = lo + g
                nc.gpsimd.indirect_dma_start(
                    out=g_tile[:, lo * D : hi * D],
                    out_offset=None,
                    in_=centers[:],
                    in_offset=bass.IndirectOffsetOnAxis(
                        ap=lab_tile[:, lo:hi], axis=0
                    ),
                )
                lo = hi

            # x on an HW ring, in parallel with labels
            x_tile = io_pool.tile([P, J * D], bf16)
            x_eng = getattr(nc, X_ENGINE)
            cw = J // X_CHUNKS * D
            for i in range(X_CHUNKS):
                x_eng.dma_start(
                    out=x_tile[:, i * cw : (i + 1) * cw],
                    in_=x[:, i * cw : (i + 1) * cw],
                )

            dists = io_pool.tile([P, J], mybir.dt.float32)

            for j, plan in enumerate(COMPUTE_PLAN):
                sub_eng = nc.gpsimd if plan[0] == "p" else nc.vector
                diff = work.tile([P, D], bf16, tag="d")
                sub_eng.tensor_tensor(
                    out=diff[:],
                    in0=x_tile[:, j * D : (j + 1) * D],
                    in1=g_tile[:, j * D : (j + 1) * D],
                    op=mybir.AluOpType.subtract,
                )
                sq = work.tile([P, D], bf16, tag="s")
                if plan[1] == "p":
                    raise ValueError("TensorScalarPtr unsupported on Pool")
                if plan[1] == "a":
                    nc.scalar.activation(
                        out=sq[:],
                        in_=diff[:],
                        func=mybir.ActivationFunctionType.Square,
                        accum_out=dists[:, j : j + 1],
                    )
                else:
                    nc.vector.scalar_tensor_tensor(
                        out=sq[:],
                        in0=diff[:],
                        scalar=0.0,
                        in1=diff[:],
                        op0=mybir.AluOpType.add,
                        op1=mybir.AluOpType.mult,
                        accum_out=dists[:, j : j + 1],
                    )

            nc.sync.dma_start(out=out[:], in_=dists[:])

    nc.compile()
    return nc


def _run(in_maps, trace=False):
    from concourse.bass_utils import run_bass_kernel_spmd

    if "nc" not in _cache:
        _cache["nc"] = _build_nc()
    return run_bass_kernel_spmd(
        _cache["nc"], in_maps, list(range(NCORES)), trace=trace
    )


def kernel(x, labels, centers, _trace=False):
    x = np.ascontiguousarray(np.asarray(x)).astype(ml_dtypes.bfloat16)
    labels = np.asarray(labels).astype(np.int32)
    centers = np.ascontiguousarray(np.asarray(centers)).astype(ml_dtypes.bfloat16)

    R = ROWS_PER_CORE
    in_maps = []
    for c in range(NCORES):
        lo = c * R
        hi = lo + R
        in_maps.append(
            {
                "x": x[lo:hi].reshape(P, J * D),
                "labels": np.ascontiguousarray(labels[lo:hi].reshape(P, J)),
                "centers": centers,
            }
        )

    res = _run(in_maps, trace=_trace)
    dists = np.concatenate(
        [res.results[c]["out"].reshape(R) for c in range(NCORES)]
    )
    loss = np.clip(dists, CLAMP, 1.0 / CLAMP).mean(dtype=np.float64)
    out = np.asarray(loss, dtype=np.float32)
    if _trace:
        return out, res
    return out
